# revision 1
# baseline (speedup 1.0000x reference)
"""GAT self-attention Trainium2 kernel.

Full inputs -> shard graphs over 8 NeuronCores -> full output.

Math (per graph n, reference reformulated):
  g_i = sigmoid(relu(q @ W1_i) @ W2_i)            [2d]
  u_i^L = W_i @ (g_i[:d] * a_i[:d])               [k]   (left projector)
  u_i^R = W_i @ (g_i[d:] * a_i[d:])               [k]   (right projector)
  left_i = X @ u_i^L ; right_i = X @ u_i^R        [E]
  score[i,j] = lrelu(left_t[i] + right_t[j]), t = adj[i,j]; -BIG if adj==0
  E = exp(score); rs = rowsum(E); Xs = X / rs[:,None]
  out = (E^T @ Xs) @ W_2          (== softmax(score)^T @ (X @ W_2))
"""
import numpy as np
from contextlib import ExitStack

import concourse.bass as bass
import concourse.tile as tile
from concourse import mybir, bacc
from concourse.masks import make_identity

F32 = mybir.dt.float32
F32R = mybir.dt.float32r
U8 = mybir.dt.uint8
I32 = mybir.dt.int32
AF = mybir.ActivationFunctionType
OP = mybir.AluOpType

N_CORES = 8
N, E, K, D = 64, 512, 512, 512   # graphs, entities, in_dim, out_dim
NG = N // N_CORES                # graphs per core
NT = 3                           # edge types
P = 128
EC = E // P                      # 4 partition chunks of E
KC = K // P
DC2 = (2 * D) // P               # 8 chunks of the 2d gate dim
NEG_BIG = -200.0
LRELU_SLOPE = 0.2
USE_HW_LRELU = True   # ACT Lrelu not implemented in CoreSim; set False for sim runs



def _dma_split(nc, dst, src, pieces):
    """Split a big load along the leading src dim across sync/scalar queues."""
    n0 = dst.shape[1]
    step = max(1, n0 // pieces)
    engs = [nc.sync, nc.scalar]
    i = 0
    c = 0
    while i < n0:
        j = min(n0, i + step)
        engs[c % 2].dma_start(dst[:, i:j], src[:, i:j])
        i = j
        c += 1

def build(nc, reps=1):
    x = nc.dram_tensor("x", [NG, E, K], F32R, kind="ExternalInput").ap()
    adj = nc.dram_tensor("adj", [NG, E, E], I32, kind="ExternalInput").ap()
    qv = nc.dram_tensor("qv", [NG, K], F32R, kind="ExternalInput").ap()
    Wt = nc.dram_tensor("Wt", [NT, K, D], F32R, kind="ExternalInput").ap()
    at = nc.dram_tensor("at", [NT, 2 * D], F32, kind="ExternalInput").ap()
    W1 = nc.dram_tensor("W1", [NT, K, 2 * D], F32R, kind="ExternalInput").ap()
    W2q = nc.dram_tensor("W2q", [NT, 2 * D, 2 * D], F32R, kind="ExternalInput").ap()
    out = nc.dram_tensor("out", [NG, E, D], F32, kind="ExternalOutput").ap()
    nc._gat_io = (x, adj, qv, Wt, at, W1, W2q, out)

    _build_once(nc, reps)


def _build_once(nc, reps=1):
    x, adj, qv, Wt, at, W1, W2q, out = nc._gat_io
    with tile.TileContext(nc) as tc, ExitStack() as ctx:
        # ---------------- persistent pools ----------------
        pers = ctx.enter_context(tc.tile_pool(name="pers", bufs=1))
        ident = pers.tile([P, P], F32)
        make_identity(nc, ident[:])
        ones_stage = pers.tile([1, E], F32)
        nc.vector.memset(ones_stage[:], 1.0)
        ones_row = pers.tile([1, E], F32R)
        nc.vector.tensor_copy(ones_row[:], ones_stage[:])
        neg_col = pers.tile([P, 1], F32)
        nc.vector.memset(neg_col[:], NEG_BIG)
        # U_all[k%128, kc, c, n]: c in 0..2 -> left type c+1, 3..5 -> right
        U_all = pers.tile([P, KC, 2 * NT, NG], F32R)
        Wt2_sb = pers.tile([P, KC, D], F32R)
        _dma_split(nc, Wt2_sb[:], Wt[2].rearrange("(c p) d -> p c d", p=P), 2)

        # ---------------- prep phase ----------------
        def run_prep():
          with tc.tile_pool(name="prep", bufs=1) as prep:
            # qT[k%128, kc, n] via PE transposes of the natural [NG, K] layout
            qv_nat = prep.tile([NG, K], F32R)
            nc.sync.dma_start(qv_nat[:], qv)
            qT = prep.tile([P, KC, NG], F32R)
            for kc in range(KC):
                qps = ps_v.tile([P, NG], F32, tag="v")
                nc.tensor.transpose(
                    qps[:], qv_nat[:, kc * P:(kc + 1) * P].bitcast(F32), ident[:NG, :NG])
                nc.vector.tensor_copy(qT[:, kc, :], qps[:])
            # aT[d2%128, dc2]  (2d = 1024)
            aT = prep.tile([P, DC2, NT], F32)
            with nc.allow_non_contiguous_dma(reason="small aT load"):
                for t in range(NT):
                    nc.sync.dma_start(aT[:, :, t:t + 1],
                                      at[t].rearrange("(c p) -> p c", p=P)[:, :, None])

            for i in range(NT):
                # rrT = relu(W1_i^T @ qT): [2d, NG] laid out [128, DC2, NG]
                rrT = prep.tile([P, DC2, NG], F32R, tag="rrT")
                for whalf in range(2):
                    W1_sb = prep.tile([P, KC, D], F32R, tag="w1")
                    _dma_split(nc, W1_sb[:],
                               W1[i, :, whalf * D:(whalf + 1) * D].rearrange(
                                   "(c p) f -> p c f", p=P), 4)
                    for oc in range(DC2 // 2):
                        oc_g = whalf * (DC2 // 2) + oc
                        pps = ps_v.tile([P, NG], F32, tag="v")
                        for kc in range(KC):
                            nc.tensor.matmul(
                                pps[:], W1_sb[:, kc, oc * P:(oc + 1) * P],
                                qT[:, kc, :],
                                start=(kc == 0), stop=(kc == KC - 1))
                        nc.scalar.activation(rrT[:, oc_g, :], pps[:], AF.Relu)
                # gT = sigmoid(W2q_i^T @ rrT), W2q loaded in two out-halves
                gvT = prep.tile([P, DC2, NG], F32, tag="gvT")
                for half in range(2):
                    W2_sb = prep.tile([P, DC2, D], F32R, tag="w2")
                    _dma_split(
                        nc, W2_sb[:],
                        W2q[i, :, half * D:(half + 1) * D].rearrange(
                            "(c p) f -> p c f", p=P), 4)
                    for oc in range(DC2 // 2):
                        oc_g = half * (DC2 // 2) + oc
                        pps = ps_v.tile([P, NG], F32, tag="v")
                        for dc in range(DC2):
                            nc.tensor.matmul(
                                pps[:], W2_sb[:, dc, oc * P:(oc + 1) * P],
                                rrT[:, dc, :],
                                start=(dc == 0), stop=(dc == DC2 - 1))
                        nc.scalar.activation(gvT[:, oc_g, :], pps[:], AF.Sigmoid)
                # vT = gT * aT_i  (per-element over the 2d axis, bcast over n)
                vT = prep.tile([P, DC2, NG], F32R, tag="vT")
                nc.vector.tensor_tensor(
                    vT[:], gvT[:], aT[:, :, i:i + 1].broadcast_to((P, DC2, NG)),
                    OP.mult)
                # WT_i = W_i^T via PE transposes: [d%128, dc, k]
                W_sb = prep.tile([P, KC, D], F32R, tag="wsb")
                _dma_split(nc, W_sb[:], Wt[i].rearrange("(c p) d -> p c d", p=P), 2)
                WTi = prep.tile([P, EC, K], F32R, tag="wti")
                for dc in range(EC):
                    tps = ps_tr.tile([P, E], F32, tag="tr")
                    for kc in range(KC):
                        nc.tensor.transpose(
                            tps[:, kc * P:(kc + 1) * P],
                            W_sb[:, kc, dc * P:(dc + 1) * P].bitcast(F32), ident[:])
                    nc.vector.tensor_copy(WTi[:, dc, :], tps[:])
                # U_i(side) = W_i @ v-half : contraction over d
                for s in range(2):
                    ups = ps_v.tile([P, KC, NG], F32, tag="v")
                    for kc in range(KC):
                        for dc in range(EC):
                            nc.tensor.matmul(
                                ups[:, kc, :],
                                WTi[:, dc, kc * P:(kc + 1) * P],
                                vT[:, s * EC + dc, :],
                                start=(dc == 0), stop=(dc == EC - 1))
                    # c index: left types at 0..2, right at 3..5 (c = 3*s + i)
                    nc.vector.tensor_copy(U_all[:, :, 3 * s + i, :], ups[:])

        # ---------------- main per-graph pipeline ----------------
        sbuf = ctx.enter_context(tc.tile_pool(name="sbuf", bufs=2))
        deep = ctx.enter_context(tc.tile_pool(name="deep", bufs=3))
        small = ctx.enter_context(tc.tile_pool(name="small", bufs=2))
        one = ctx.enter_context(tc.tile_pool(name="one", bufs=1))
        ps_big = ctx.enter_context(tc.tile_pool(name="ps_big", bufs=2, space="PSUM"))
        ps_v = ctx.enter_context(tc.tile_pool(name="ps_v", bufs=4, space="PSUM"))
        ps_tr = ctx.enter_context(tc.tile_pool(name="ps_tr", bufs=1, space="PSUM"))
        ps_lr = ctx.enter_context(tc.tile_pool(name="ps_lr", bufs=1, space="PSUM"))

        def phase1(n):
            """front half: inputs, Xt, LR rows, stacks, masks"""
            X_sb = deep.tile([P, EC, K], F32R, tag="X")
            nc.sync.dma_start(X_sb[:, 0:2], x[n].rearrange("(c p) k -> p c k", p=P)[:, 0:2])
            nc.scalar.dma_start(X_sb[:, 2:4], x[n].rearrange("(c p) k -> p c k", p=P)[:, 2:4])
            adj_sb = sbuf.tile([P, EC, E], I32, tag="adj")
            nc.scalar.dma_start(adj_sb[:, 0:2], adj[n].rearrange("(c p) j -> p c j", p=P)[:, 0:2])
            nc.sync.dma_start(adj_sb[:, 2:4], adj[n].rearrange("(c p) j -> p c j", p=P)[:, 2:4])

            Xt_sb = sbuf.tile([P, KC, E], F32R, tag="Xt")
            for kc in range(KC):
                tps = ps_tr.tile([P, E], F32, tag="tr")
                for ec in range(EC):
                    nc.tensor.transpose(
                        tps[:, ec * P:(ec + 1) * P],
                        X_sb[:, ec, kc * P:(kc + 1) * P].bitcast(F32), ident[:])
                nc.scalar.copy(Xt_sb[:, kc, :], tps[:])

            pLR = ps_lr.tile([2 * NT, E], F32, tag="lr")
            for kc in range(KC):
                nc.tensor.matmul(pLR[:], U_all[:, kc, :, n], Xt_sb[:, kc, :],
                                 start=(kc == 0), stop=(kc == KC - 1))
            LR_sb = small.tile([2 * NT, E], F32R, tag="lrs")
            nc.scalar.copy(LR_sb[:], pLR[:])

            lhsT = []
            rhsT = []
            for t in range(NT):
                eng_a = nc.sync if t % 2 == 0 else nc.scalar
                eng_b = nc.scalar if t % 2 == 0 else nc.sync
                lt = small.tile([2, E], F32R, tag=f"lt{t}")
                eng_a.dma_start(lt[0:1, :], ones_row[:])
                eng_b.dma_start(lt[1:2, :], LR_sb[t:t + 1, :])
                rt = small.tile([2, E], F32R, tag=f"rt{t}")
                eng_a.dma_start(rt[0:1, :], LR_sb[NT + t:NT + t + 1, :])
                eng_b.dma_start(rt[1:2, :], ones_row[:])
                lhsT.append(lt)
                rhsT.append(rt)

            m0 = sbuf.tile([P, EC, E], U8, tag="m0")
            m2 = sbuf.tile([P, EC, E], U8, tag="m2")
            m3 = sbuf.tile([P, EC, E], U8, tag="m3")
            for h in range(2):
                sl = slice(2 * h, 2 * h + 2)
                nc.gpsimd.tensor_scalar(m2[:, sl], adj_sb[:, sl], 2, None, OP.is_equal)
                nc.gpsimd.tensor_scalar(m3[:, sl], adj_sb[:, sl], 3, None, OP.is_equal)
                nc.gpsimd.tensor_scalar(m0[:, sl], adj_sb[:, sl], 0, None, OP.is_equal)
            return dict(X_sb=X_sb, lhsT=lhsT, rhsT=rhsT, m0=m0, m2=m2, m3=m3)

        def phase2(n, st):
            """back half: select, exp, F, out"""
            X_sb = st["X_sb"]; lhsT = st["lhsT"]; rhsT = st["rhsT"]
            m0 = st["m0"]; m2 = st["m2"]; m3 = st["m3"]
            E_sb = deep.tile([P, EC, E], F32R, tag="E")
            rs = small.tile([P, EC], F32, tag="rs")
            for ic in range(EC):
                pv = []
                for t in range(NT):
                    pvt = ps_v.tile([P, E], F32, tag="v")
                    nc.tensor.matmul(pvt[:], lhsT[t][:, ic * P:(ic + 1) * P],
                                     rhsT[t][:], start=True, stop=True)
                    pv.append(pvt)
                nc.vector.copy_predicated(pv[0][:], m2[:, ic, :], pv[1][:])
                nc.vector.copy_predicated(pv[0][:], m3[:, ic, :], pv[2][:])
                nc.vector.copy_predicated(pv[0][:], m0[:, ic, :],
                                          neg_col[:, 0:1].broadcast_to((P, E)))
                ab = small.tile([P, E], F32, tag="ab")
                nc.scalar.activation(ab[:], pv[0][:], AF.Abs, scale=0.4)
                sc = small.tile([P, E], F32, tag="sc")
                nc.vector.scalar_tensor_tensor(sc[:], pv[0][:], 0.6, ab[:],
                                               OP.mult, OP.add)
                nc.scalar.activation(E_sb[:, ic, :], sc[:], AF.Exp,
                                     accum_out=rs[:, ic:ic + 1])
                rsr_ic = small.tile([P, EC], F32, tag="rsr")
                nc.vector.reciprocal(rsr_ic[:, ic:ic + 1], rs[:, ic:ic + 1])
                nc.vector.tensor_scalar(E_sb[:, ic, :], E_sb[:, ic, :].bitcast(F32),
                                        rsr_ic[:, ic:ic + 1], None, OP.mult)

            F_sb = sbuf.tile([P, KC, E], F32R, tag="F")
            for kc in range(KC):
                pF = ps_big.tile([P, E], F32, tag="big")
                for ec in range(EC):
                    nc.tensor.matmul(pF[:], X_sb[:, ec, kc * P:(kc + 1) * P],
                                     E_sb[:, ec, :],
                                     start=(ec == 0), stop=(ec == EC - 1))
                nc.scalar.copy(F_sb[:, kc, :], pF[:])

            for jc in range(EC):
                pO = ps_big.tile([P, D], F32, tag="big")
                for kc in range(KC):
                    nc.tensor.matmul(pO[:], F_sb[:, kc, jc * P:(jc + 1) * P],
                                     Wt2_sb[:, kc, :],
                                     start=(kc == 0), stop=(kc == KC - 1))
                o_sb = small.tile([P, D], F32, tag="osb")
                nc.scalar.copy(o_sb[:], pO[:])
                (nc.sync if jc % 2 == 0 else nc.scalar).dma_start(
                    out[n, jc * P:(jc + 1) * P, :], o_sb[:])

        def body_all(_iv=None):
          run_prep()
          for n in range(NG):
              phase2(n, phase1(n))

        if reps == 1:
            body_all()
        else:
            with tc.For_i(0, reps, 1) as _iv:
                body_all(_iv)
    return nc


_NC_CACHE = {}
TRACE = False
_LAST = {}


def _get_nc():
    if "nc" not in _NC_CACHE:
        nc = bacc.Bacc("TRN2", target_bir_lowering=False, debug=False)
        build(nc)
        nc.compile()
        _NC_CACHE["nc"] = nc
    return _NC_CACHE["nc"]


def kernel(input_state, adj, entity_mask, query_vec, W_type, a_type,
           qattn_W1, qattn_W2):
    from concourse import bass_utils
    nc = _get_nc()
    input_state = np.ascontiguousarray(input_state, dtype=np.float32)
    adj = np.ascontiguousarray(adj, dtype=np.int32)
    query_vec = np.ascontiguousarray(query_vec, dtype=np.float32)
    W_type = np.ascontiguousarray(W_type, dtype=np.float32)
    a_type = np.ascontiguousarray(a_type, dtype=np.float32)
    qattn_W1 = np.ascontiguousarray(qattn_W1, dtype=np.float32)
    qattn_W2 = np.ascontiguousarray(qattn_W2, dtype=np.float32)

    in_maps = []
    for c in range(N_CORES):
        sl = slice(c * NG, (c + 1) * NG)
        in_maps.append({
            "x": input_state[sl], "adj": adj[sl], "qv": query_vec[sl],
            "Wt": W_type, "at": a_type, "W1": qattn_W1, "W2q": qattn_W2,
        })
    res = bass_utils.run_bass_kernel_spmd(nc, in_maps, core_ids=list(range(N_CORES)),
                                          trace=TRACE, stitch_traces=TRACE)
    _LAST["exec_ns"] = res.exec_time_ns
    _LAST["mean_ns"] = res.mean_exec_time_ns
    _LAST["trace"] = res.instructions_and_trace
    _LAST["scope_times"] = res.per_core_scope_times
    out = np.concatenate([r["out"] for r in res.results], axis=0)
    return out.astype(np.float32)



# revision 7
# speedup vs baseline: 1.3835x; 1.3835x over previous
"""GAT self-attention Trainium2 kernel (v2, bf16 data path).

Full inputs -> shard graphs over 8 NeuronCores -> full output.

Math (per graph n, reference reformulated):
  g_i = sigmoid(relu(q @ W1_i) @ W2_i)            [2d]
  u_i^L = W_i @ (g_i[:d] * a_i[:d])               [k]   (left projector)
  u_i^R = W_i @ (g_i[d:] * a_i[d:])               [k]   (right projector)
  l_i = X @ u_i^L ; r_i = X @ u_i^R               [E]
  S[i,j] = lrelu(l_t[i] + r_t[j]), t = adj[i,j]
  E' = exp(S) * (adj > 0); rs = rowsum(E')
  h = X @ W_2 ; hs = h / rs[:, None]
  out = E'^T @ hs          (== softmax(scores)^T @ (X @ W_2))

Key implementation points:
  - everything bf16 except the score rank-2 matmuls (f32r) and PSUM.
  - adj shipped as bf16 so type masks are DVE tensor_scalar is_equal in
    4x mode; no gpsimd is_equal, no int32 adj DMA.
  - one DMA per tensor (HWDGE slot costs ~630ns per DMA instruction).
  - scores: per type a single rank-2 matmul from a persistent 12-row
    stack [1s, l_t, r_t, 1s] built by one stt pass from the LR matmul.
  - type select via 2 copy_predicated; adj==0 handled by multiplying
    exp by (adj>0) in the same stt pass that row-sums E'.
  - softmax normalization folded into h's PSUM->SBUF copy (scale by
    1/rs), so no extra pass over the [E,E] matrix.
"""
import numpy as np
from contextlib import ExitStack

import concourse.bass as bass
import concourse.tile as tile
from concourse import mybir, bacc
from concourse.masks import make_identity

F32 = mybir.dt.float32
F32R = mybir.dt.float32r
BF16 = mybir.dt.bfloat16
AF = mybir.ActivationFunctionType
OP = mybir.AluOpType

N_CORES = 8
N, E, K, D = 64, 512, 512, 512   # graphs, entities, in_dim, out_dim
NG = N // N_CORES                # graphs per core
NT = 3                           # edge types
P = 128
EC = E // P                      # 4 partition chunks of E
KC = K // P
DC2 = (2 * D) // P               # 8 chunks of the 2d gate dim


def build(nc, reps=1):
    x = nc.dram_tensor("x", [NG, E, K], BF16, kind="ExternalInput").ap()
    adjf = nc.dram_tensor("adjf", [NG, E, E], BF16, kind="ExternalInput").ap()
    qv = nc.dram_tensor("qv", [NG, K], BF16, kind="ExternalInput").ap()
    Wt = nc.dram_tensor("Wt", [NT, K, D], BF16, kind="ExternalInput").ap()
    at = nc.dram_tensor("at", [NT, 2 * D], F32, kind="ExternalInput").ap()
    W1 = nc.dram_tensor("W1", [NT, K, 2 * D], BF16, kind="ExternalInput").ap()
    W2q = nc.dram_tensor("W2q", [NT, 2 * D, 2 * D], BF16, kind="ExternalInput").ap()
    out = nc.dram_tensor("out", [NG, E, D], BF16, kind="ExternalOutput").ap()
    nc._gat_io = (x, adjf, qv, Wt, at, W1, W2q, out)
    _build_once(nc, reps)


def _build_once(nc, reps=1):
    x, adjf, qv, Wt, at, W1, W2q, out = nc._gat_io
    with tile.TileContext(nc) as tc, ExitStack() as ctx:
        # ---------------- persistent tiles ----------------
        pers = ctx.enter_context(tc.tile_pool(name="pers", bufs=1))
        identb = pers.tile([P, P], BF16)
        make_identity(nc, identb[:])
        identf = pers.tile([P, P], F32)
        make_identity(nc, identf[:])
        # U6[k%128, kc, c, n]: c in 0..2 -> left type c, 3..5 -> right
        U6 = pers.tile([P, KC, 2 * NT, NG], BF16)
        Wt2_sb = pers.tile([P, KC, D], BF16)
        nc.sync.dma_start(Wt2_sb[:], Wt[2].rearrange("(c p) d -> p c d", p=P))
        aT = pers.tile([P, DC2, NT], F32)
        # Score-stack tiles: operand pair for type t at base partition 32*t
        # (PE requires matmul operands to start at partition 0/32/64).
        # tileL rows 32t..32t+1 = [1s; l_t]; tileR rows 32t..32t+1 = [r_t; 1s].
        # The ones rows are written once; l/r rows are refreshed per graph
        # via a partition-strided copy.  A/B double buffering across graphs.
        stkL = [pers.tile([66, E], F32, name=f"stkL{i}") for i in range(2)]
        stkR = [pers.tile([66, E], F32, name=f"stkR{i}") for i in range(2)]
        for s in range(2):
            for t in range(NT):
                nc.vector.memset(stkL[s][32 * t:32 * t + 1, :], 1.0)
                nc.vector.memset(stkR[s][32 * t + 1:32 * t + 2, :], 1.0)

        # ---------------- PSUM pools (8 banks total) ----------------
        ps_s1 = ctx.enter_context(tc.tile_pool(name="ps_s1", bufs=2, space="PSUM"))
        ps_s23 = ctx.enter_context(tc.tile_pool(name="ps_s23", bufs=1, space="PSUM"))
        ps_big = ctx.enter_context(tc.tile_pool(name="ps_big", bufs=2, space="PSUM"))
        ps_tr = ctx.enter_context(tc.tile_pool(name="ps_tr", bufs=1, space="PSUM"))
        ps_lr = ctx.enter_context(tc.tile_pool(name="ps_lr", bufs=1, space="PSUM"))

        # ---------------- prep: gates + projector vectors ----------------
        def run_prep():
          with tc.tile_pool(name="prep", bufs=2) as prep:
            qv_nat = prep.tile([NG, K], BF16, tag="qn")
            nc.sync.dma_start(qv_nat[:], qv)
            at_nat = prep.tile([NT, 2 * D], F32, tag="an")
            nc.sync.dma_start(at_nat[:], at)
            # qT[k%128, kc, n]
            qT = prep.tile([P, KC, NG], BF16, tag="qT")
            for kc in range(KC):
                tps = ps_tr.tile([P, E], BF16, tag="tr")
                nc.tensor.transpose(
                    tps[:, 0:NG], qv_nat[:, kc * P:(kc + 1) * P], identb[:NG, :NG])
                nc.vector.tensor_copy(qT[:, kc, :], tps[:, 0:NG])
            # aT[d2%128, dc2, t] via PE transposes of [3, 2D]
            for oc in range(DC2):
                aps = ps_s1.tile([P, E], F32, tag="s1")
                nc.tensor.transpose(
                    aps[:, 0:NT], at_nat[:, oc * P:(oc + 1) * P], identf[:NT, :NT])
                nc.vector.tensor_copy(aT[:, oc, :], aps[:, 0:NT])

            for i in range(NT):
                # rrT[2d%128, oc, n] = relu(W1_i^T q)^T
                W1_sb = prep.tile([P, KC, 2 * D], BF16, tag="w1")
                nc.sync.dma_start(
                    W1_sb[:], W1[i].rearrange("(c p) f -> p c f", p=P))
                rrT = prep.tile([P, DC2, NG], BF16, tag="rrT")
                for oc in range(DC2):
                    pps = ps_s1.tile([P, E], F32, tag="s1")
                    for kc in range(KC):
                        nc.tensor.matmul(
                            pps[:, 0:NG], W1_sb[:, kc, oc * P:(oc + 1) * P],
                            qT[:, kc, :], start=(kc == 0), stop=(kc == KC - 1))
                    nc.scalar.activation(rrT[:, oc, :], pps[:, 0:NG], AF.Relu)
                # gvT = sigmoid(W2q_i^T rrT)
                W2_sb = prep.tile([P, DC2, 2 * D], BF16, tag="w2")
                nc.sync.dma_start(
                    W2_sb[:, :, 0:D],
                    W2q[i, :, 0:D].rearrange("(c p) f -> p c f", p=P))
                nc.sync.dma_start(
                    W2_sb[:, :, D:2 * D],
                    W2q[i, :, D:2 * D].rearrange("(c p) f -> p c f", p=P))
                gvT = prep.tile([P, DC2, NG], BF16, tag="gvT")
                for oc in range(DC2):
                    pps = ps_s1.tile([P, E], F32, tag="s1")
                    for dc in range(DC2):
                        nc.tensor.matmul(
                            pps[:, 0:NG], W2_sb[:, dc, oc * P:(oc + 1) * P],
                            rrT[:, dc, :], start=(dc == 0), stop=(dc == DC2 - 1))
                    nc.scalar.activation(gvT[:, oc, :], pps[:, 0:NG], AF.Sigmoid)
                # vT = gvT * a_i (broadcast over n)
                vT = prep.tile([P, DC2, NG], BF16, tag="vT")
                nc.vector.tensor_tensor(
                    vT[:], gvT[:], aT[:, :, i:i + 1].broadcast_to((P, DC2, NG)),
                    OP.mult)
                # WTi = W_i^T
                W_sb = prep.tile([P, KC, D], BF16, tag="wsb")
                nc.sync.dma_start(W_sb[:], Wt[i].rearrange("(c p) d -> p c d", p=P))
                WTi = prep.tile([P, EC, K], BF16, tag="wti")
                for dc in range(EC):
                    tps = ps_tr.tile([P, E], BF16, tag="tr")
                    for kc in range(KC):
                        nc.tensor.transpose(
                            tps[:, kc * P:(kc + 1) * P],
                            W_sb[:, kc, dc * P:(dc + 1) * P], identb[:])
                    nc.scalar.copy(WTi[:, dc, :], tps[:])
                # U columns: left at c=i, right at c=3+i
                for s in range(2):
                    pu = ps_s23.tile([P, 2, E], F32, tag="s23")
                    for kc in range(KC):
                        for dc in range(EC):
                            nc.tensor.matmul(
                                pu[:, 0, kc * NG:(kc + 1) * NG],
                                WTi[:, dc, kc * P:(kc + 1) * P],
                                vT[:, s * EC + dc, :],
                                start=(dc == 0), stop=(dc == EC - 1))
                    for kc in range(KC):
                        nc.vector.tensor_copy(
                            U6[:, kc, 3 * s + i, :],
                            pu[:, 0, kc * NG:(kc + 1) * NG])

        # ---------------- main per-graph pipeline ----------------
        deep = ctx.enter_context(tc.tile_pool(name="deep", bufs=2))
        sbuf = ctx.enter_context(tc.tile_pool(name="sbuf", bufs=2))
        small = ctx.enter_context(tc.tile_pool(name="small", bufs=2))

        def do_graph(n):
            # ---- loads ----
            X_sb = deep.tile([P, EC, K], BF16, tag="X")
            nc.sync.dma_start(X_sb[:], x[n].rearrange("(c p) k -> p c k", p=P))
            adj_sb = deep.tile([P, EC, E], BF16, tag="adj")
            nc.sync.dma_start(adj_sb[:], adjf[n].rearrange("(c p) j -> p c j", p=P))

            # ---- masks (DVE 4x on bf16) ----
            mz = sbuf.tile([P, EC, E], BF16, tag="mz")
            nc.vector.tensor_scalar(mz[:], adj_sb[:], 0.5, None, OP.is_gt)
            m2 = sbuf.tile([P, EC, E], BF16, tag="m2")
            nc.vector.tensor_scalar(m2[:], adj_sb[:], 2.0, None, OP.is_equal)
            m3 = sbuf.tile([P, EC, E], BF16, tag="m3")
            nc.vector.tensor_scalar(m3[:], adj_sb[:], 3.0, None, OP.is_equal)

            # ---- Xt via PE transposes (copies on Pool) ----
            Xt_sb = sbuf.tile([P, KC, E], BF16, tag="Xt")
            for kc in range(KC):
                tps = ps_tr.tile([P, E], BF16, tag="tr")
                for ec in range(EC):
                    nc.tensor.transpose(
                        tps[:, ec * P:(ec + 1) * P],
                        X_sb[:, ec, kc * P:(kc + 1) * P], identb[:])
                nc.gpsimd.tensor_copy(Xt_sb[:, kc, :], tps[:])

            # ---- LR rows -> score stacks ----
            pLR = ps_lr.tile([2 * NT, E], F32, tag="lr")
            for kc in range(KC):
                nc.tensor.matmul(pLR[:], U6[:, kc, :, n], Xt_sb[:, kc, :],
                                 start=(kc == 0), stop=(kc == KC - 1))
            sL, sR = stkL[n % 2], stkR[n % 2]
            # l_t -> row 32t+1 of stkL; r_t -> row 32t of stkR
            nc.vector.tensor_copy(sL[1:66:32, :], pLR[0:NT, :])
            nc.vector.tensor_copy(sR[0:65:32, :], pLR[NT:2 * NT, :])

            # ---- per-chunk scores + h ----
            rs = small.tile([P, EC], F32, tag="rs")
            rsr = small.tile([P, EC], F32, tag="rsr")
            E_sb = deep.tile([P, EC, E], BF16, tag="E")
            hs_sb = deep.tile([P, EC, D], BF16, tag="hs")
            for ic in range(EC):
                pv1 = ps_s1.tile([P, E], F32, tag="s1")
                nc.tensor.matmul(
                    pv1[:], sL[0:2, ic * P:(ic + 1) * P].bitcast(F32R),
                    sR[0:2, :].bitcast(F32R), start=True, stop=True)
                pv23 = ps_s23.tile([P, 2, E], F32, tag="s23")
                nc.tensor.matmul(
                    pv23[:, 0, :], sL[32:34, ic * P:(ic + 1) * P].bitcast(F32R),
                    sR[32:34, :].bitcast(F32R), start=True, stop=True)
                nc.tensor.matmul(
                    pv23[:, 1, :], sL[64:66, ic * P:(ic + 1) * P].bitcast(F32R),
                    sR[64:66, :].bitcast(F32R), start=True, stop=True)
                nc.vector.copy_predicated(pv1[:], m2[:, ic, :], pv23[:, 0, :])
                nc.vector.copy_predicated(pv1[:], m3[:, ic, :], pv23[:, 1, :])
                lr_sb = small.tile([P, E], BF16, tag="lrl")
                nc.scalar.activation(lr_sb[:], pv1[:], AF.Lrelu, alpha=0.2)
                e1_sb = small.tile([P, E], BF16, tag="e1")
                nc.scalar.activation(e1_sb[:], lr_sb[:], AF.Exp)
                # E' = e1 * (adj>0), rowsum into rs  (DVE 4x)
                nc.vector.scalar_tensor_tensor(
                    E_sb[:, ic, :], e1_sb[:], 1.0, mz[:, ic, :],
                    OP.mult, OP.mult, accum_out=rs[:, ic:ic + 1])
                nc.vector.reciprocal(rsr[:, ic:ic + 1], rs[:, ic:ic + 1])
                # h chunk; normalization folded into the PSUM->SBUF copy
                pH = ps_big.tile([P, D], F32, tag="big")
                for kc in range(KC):
                    nc.tensor.matmul(pH[:], Xt_sb[:, kc, ic * P:(ic + 1) * P],
                                     Wt2_sb[:, kc, :],
                                     start=(kc == 0), stop=(kc == KC - 1))
                nc.gpsimd.tensor_scalar(hs_sb[:, ic, :], pH[:],
                                        rsr[:, ic:ic + 1], None, OP.mult)

            # ---- out = E'^T @ hs ----
            out_sb = sbuf.tile([P, EC, D], BF16, tag="osb")
            for jc in range(EC):
                pO = ps_big.tile([P, D], F32, tag="big")
                for ic in range(EC):
                    nc.tensor.matmul(pO[:], E_sb[:, ic, jc * P:(jc + 1) * P],
                                     hs_sb[:, ic, :],
                                     start=(ic == 0), stop=(ic == EC - 1))
                nc.scalar.copy(out_sb[:, jc, :], pO[:])
            nc.sync.dma_start(out[n].rearrange("(c p) d -> p c d", p=P), out_sb[:])

        def body_all(_iv=None):
            run_prep()
            for n in range(NG):
                do_graph(n)

        if reps == 1:
            body_all()
        else:
            with tc.For_i(0, reps, 1) as _iv:
                body_all(_iv)
    return nc


_NC_CACHE = {}
TRACE = False
_LAST = {}


def _get_nc():
    if "nc" not in _NC_CACHE:
        nc = bacc.Bacc("TRN2", target_bir_lowering=False, debug=False)
        build(nc)
        nc.compile()
        _NC_CACHE["nc"] = nc
    return _NC_CACHE["nc"]


def kernel(input_state, adj, entity_mask, query_vec, W_type, a_type,
           qattn_W1, qattn_W2):
    from concourse import bass_utils
    import ml_dtypes
    bf16 = ml_dtypes.bfloat16
    nc = _get_nc()
    x = np.ascontiguousarray(input_state).astype(bf16)
    adjf = np.ascontiguousarray(adj).astype(bf16)
    qv = np.ascontiguousarray(query_vec).astype(bf16)
    Wt = np.ascontiguousarray(W_type).astype(bf16)
    at = np.ascontiguousarray(a_type, dtype=np.float32)
    W1 = np.ascontiguousarray(qattn_W1).astype(bf16)
    W2q = np.ascontiguousarray(qattn_W2).astype(bf16)

    in_maps = []
    for c in range(N_CORES):
        sl = slice(c * NG, (c + 1) * NG)
        in_maps.append({
            "x": x[sl], "adjf": adjf[sl], "qv": qv[sl],
            "Wt": Wt, "at": at, "W1": W1, "W2q": W2q,
        })
    res = bass_utils.run_bass_kernel_spmd(nc, in_maps, core_ids=list(range(N_CORES)),
                                          trace=TRACE, stitch_traces=TRACE)
    _LAST["exec_ns"] = res.exec_time_ns
    _LAST["mean_ns"] = res.mean_exec_time_ns
    _LAST["trace"] = res.instructions_and_trace
    out = np.concatenate([r["out"] for r in res.results], axis=0)
    return out.astype(np.float32)


# revision 9
# speedup vs baseline: 1.5520x; 1.1218x over previous
"""GAT self-attention Trainium2 kernel (v2, bf16 data path).

Full inputs -> shard graphs over 8 NeuronCores -> full output.

Math (per graph n, reference reformulated):
  g_i = sigmoid(relu(q @ W1_i) @ W2_i)            [2d]
  u_i^L = W_i @ (g_i[:d] * a_i[:d])               [k]   (left projector)
  u_i^R = W_i @ (g_i[d:] * a_i[d:])               [k]   (right projector)
  l_i = X @ u_i^L ; r_i = X @ u_i^R               [E]
  S[i,j] = lrelu(l_t[i] + r_t[j]), t = adj[i,j]
  E' = exp(S) * (adj > 0); rs = rowsum(E')
  h = X @ W_2 ; hs = h / rs[:, None]
  out = E'^T @ hs          (== softmax(scores)^T @ (X @ W_2))

Key implementation points:
  - everything bf16 except the score rank-2 matmuls (f32r) and PSUM.
  - adj shipped as bf16 so type masks are DVE tensor_scalar is_equal in
    4x mode; no gpsimd is_equal, no int32 adj DMA.
  - one DMA per tensor (HWDGE slot costs ~630ns per DMA instruction).
  - scores: per type a single rank-2 matmul from a persistent 12-row
    stack [1s, l_t, r_t, 1s] built by one stt pass from the LR matmul.
  - type select via 2 copy_predicated; adj==0 handled by multiplying
    exp by (adj>0) in the same stt pass that row-sums E'.
  - softmax normalization folded into h's PSUM->SBUF copy (scale by
    1/rs), so no extra pass over the [E,E] matrix.
"""
import numpy as np
from contextlib import ExitStack

import concourse.bass as bass
import concourse.tile as tile
from concourse import mybir, bacc
from concourse.masks import make_identity

F32 = mybir.dt.float32
F32R = mybir.dt.float32r
BF16 = mybir.dt.bfloat16
AF = mybir.ActivationFunctionType
OP = mybir.AluOpType

N_CORES = 8
N, E, K, D = 64, 512, 512, 512   # graphs, entities, in_dim, out_dim
NG = N // N_CORES                # graphs per core
NT = 3                           # edge types
P = 128
EC = E // P                      # 4 partition chunks of E
KC = K // P
DC2 = (2 * D) // P               # 8 chunks of the 2d gate dim


def build(nc, reps=1):
    x = nc.dram_tensor("x", [NG, E, K], BF16, kind="ExternalInput").ap()
    adjf = nc.dram_tensor("adjf", [NG, E, E], BF16, kind="ExternalInput").ap()
    qv = nc.dram_tensor("qv", [NG, K], BF16, kind="ExternalInput").ap()
    Wt = nc.dram_tensor("Wt", [NT, K, D], BF16, kind="ExternalInput").ap()
    at = nc.dram_tensor("at", [NT, 2 * D], F32, kind="ExternalInput").ap()
    W1 = nc.dram_tensor("W1", [NT, K, 2 * D], BF16, kind="ExternalInput").ap()
    W2q = nc.dram_tensor("W2q", [NT, 2 * D, 2 * D], BF16, kind="ExternalInput").ap()
    out = nc.dram_tensor("out", [NG, E, D], BF16, kind="ExternalOutput").ap()
    nc._gat_io = (x, adjf, qv, Wt, at, W1, W2q, out)
    _build_once(nc, reps)


def _build_once(nc, reps=1):
    x, adjf, qv, Wt, at, W1, W2q, out = nc._gat_io
    with tile.TileContext(nc) as tc, ExitStack() as ctx:
        # ---------------- persistent tiles ----------------
        pers = ctx.enter_context(tc.tile_pool(name="pers", bufs=1))
        identb = pers.tile([P, P], BF16)
        make_identity(nc, identb[:])
        identf = pers.tile([P, P], F32)
        make_identity(nc, identf[:])
        # U6[k%128, kc, c, n]: c in 0..2 -> left type c, 3..5 -> right
        U6 = pers.tile([P, KC, 2 * NT, NG], BF16)
        Wt2_sb = pers.tile([P, KC, D], BF16)
        nc.sync.dma_start(Wt2_sb[:], Wt[2].rearrange("(c p) d -> p c d", p=P))
        aT = pers.tile([P, DC2, NT], F32)
        # Score-stack tiles: operand pair for type t at base partition 32*t
        # (PE requires matmul operands to start at partition 0/32/64).
        # tileL rows 32t..32t+1 = [1s; l_t]; tileR rows 32t..32t+1 = [r_t; 1s].
        # The ones rows are written once; l/r rows are refreshed per graph
        # via a partition-strided copy.  A/B double buffering across graphs.
        stkL = [pers.tile([66, E], F32, name=f"stkL{i}") for i in range(2)]
        stkR = [pers.tile([66, E], F32, name=f"stkR{i}") for i in range(2)]
        for s in range(2):
            for t in range(NT):
                nc.vector.memset(stkL[s][32 * t:32 * t + 1, :], 1.0)
                nc.vector.memset(stkR[s][32 * t + 1:32 * t + 2, :], 1.0)

        # ---------------- PSUM pools (8 banks total) ----------------
        ps_s1 = ctx.enter_context(tc.tile_pool(name="ps_s1", bufs=2, space="PSUM"))
        ps_s23 = ctx.enter_context(tc.tile_pool(name="ps_s23", bufs=1, space="PSUM"))
        ps_big = ctx.enter_context(tc.tile_pool(name="ps_big", bufs=2, space="PSUM"))
        ps_tr = ctx.enter_context(tc.tile_pool(name="ps_tr", bufs=1, space="PSUM"))
        ps_lr = ctx.enter_context(tc.tile_pool(name="ps_lr", bufs=1, space="PSUM"))

        # ---------------- prep: gates + projector vectors ----------------
        def run_prep():
          with tc.tile_pool(name="prep", bufs=2) as prep:
            qv_nat = prep.tile([NG, K], BF16, tag="qn")
            nc.sync.dma_start(qv_nat[:], qv)
            at_nat = prep.tile([NT, 2 * D], F32, tag="an")
            nc.sync.dma_start(at_nat[:], at)
            # qT[k%128, kc, n]
            qT = prep.tile([P, KC, NG], BF16, tag="qT")
            for kc in range(KC):
                tps = ps_tr.tile([P, E], BF16, tag="tr")
                nc.tensor.transpose(
                    tps[:, 0:NG], qv_nat[:, kc * P:(kc + 1) * P], identb[:NG, :NG])
                nc.vector.tensor_copy(qT[:, kc, :], tps[:, 0:NG])
            # aT[d2%128, dc2, t] via PE transposes of [3, 2D]
            for oc in range(DC2):
                aps = ps_s1.tile([P, E], F32, tag="s1")
                nc.tensor.transpose(
                    aps[:, 0:NT], at_nat[:, oc * P:(oc + 1) * P], identf[:NT, :NT])
                nc.vector.tensor_copy(aT[:, oc, :], aps[:, 0:NT])

            for i in range(NT):
                # rrT[2d%128, oc, n] = relu(W1_i^T q)^T
                W1_sb = prep.tile([P, KC, 2 * D], BF16, tag="w1")
                nc.sync.dma_start(
                    W1_sb[:], W1[i].rearrange("(c p) f -> p c f", p=P))
                rrT = prep.tile([P, DC2, NG], BF16, tag="rrT")
                for oc in range(DC2):
                    pps = ps_s1.tile([P, E], F32, tag="s1")
                    for kc in range(KC):
                        nc.tensor.matmul(
                            pps[:, 0:NG], W1_sb[:, kc, oc * P:(oc + 1) * P],
                            qT[:, kc, :], start=(kc == 0), stop=(kc == KC - 1))
                    nc.scalar.activation(rrT[:, oc, :], pps[:, 0:NG], AF.Relu)
                # gvT = sigmoid(W2q_i^T rrT)
                W2_sb = prep.tile([P, DC2, 2 * D], BF16, tag="w2")
                nc.sync.dma_start(
                    W2_sb[:, :, 0:D],
                    W2q[i, :, 0:D].rearrange("(c p) f -> p c f", p=P))
                nc.sync.dma_start(
                    W2_sb[:, :, D:2 * D],
                    W2q[i, :, D:2 * D].rearrange("(c p) f -> p c f", p=P))
                gvT = prep.tile([P, DC2, NG], BF16, tag="gvT")
                for oc in range(DC2):
                    pps = ps_s1.tile([P, E], F32, tag="s1")
                    for dc in range(DC2):
                        nc.tensor.matmul(
                            pps[:, 0:NG], W2_sb[:, dc, oc * P:(oc + 1) * P],
                            rrT[:, dc, :], start=(dc == 0), stop=(dc == DC2 - 1))
                    nc.scalar.activation(gvT[:, oc, :], pps[:, 0:NG], AF.Sigmoid)
                # vT = gvT * a_i (broadcast over n)
                vT = prep.tile([P, DC2, NG], BF16, tag="vT")
                nc.vector.tensor_tensor(
                    vT[:], gvT[:], aT[:, :, i:i + 1].broadcast_to((P, DC2, NG)),
                    OP.mult)
                # WTi = W_i^T
                W_sb = prep.tile([P, KC, D], BF16, tag="wsb")
                nc.sync.dma_start(W_sb[:], Wt[i].rearrange("(c p) d -> p c d", p=P))
                WTi = prep.tile([P, EC, K], BF16, tag="wti")
                for dc in range(EC):
                    tps = ps_tr.tile([P, E], BF16, tag="tr")
                    for kc in range(KC):
                        nc.tensor.transpose(
                            tps[:, kc * P:(kc + 1) * P],
                            W_sb[:, kc, dc * P:(dc + 1) * P], identb[:])
                    nc.scalar.copy(WTi[:, dc, :], tps[:])
                # U columns: left at c=i, right at c=3+i
                for s in range(2):
                    pu = ps_s23.tile([P, 2, E], F32, tag="s23")
                    for kc in range(KC):
                        for dc in range(EC):
                            nc.tensor.matmul(
                                pu[:, 0, kc * NG:(kc + 1) * NG],
                                WTi[:, dc, kc * P:(kc + 1) * P],
                                vT[:, s * EC + dc, :],
                                start=(dc == 0), stop=(dc == EC - 1))
                    for kc in range(KC):
                        nc.vector.tensor_copy(
                            U6[:, kc, 3 * s + i, :],
                            pu[:, 0, kc * NG:(kc + 1) * NG])

        # ---------------- main per-graph pipeline ----------------
        deep = ctx.enter_context(tc.tile_pool(name="deep", bufs=2))
        sbuf = ctx.enter_context(tc.tile_pool(name="sbuf", bufs=2))
        small = ctx.enter_context(tc.tile_pool(name="small", bufs=2))

        def do_graph(n):
            # ---- loads ----
            X_sb = deep.tile([P, EC, K], BF16, tag="X")
            nc.sync.dma_start(X_sb[:], x[n].rearrange("(c p) k -> p c k", p=P))
            adj_sb = deep.tile([P, EC, E], BF16, tag="adj")
            nc.sync.dma_start(adj_sb[:], adjf[n].rearrange("(c p) j -> p c j", p=P))

            # ---- masks (DVE 4x on bf16) ----
            mz = sbuf.tile([P, EC, E], BF16, tag="mz")
            nc.vector.tensor_scalar(mz[:], adj_sb[:], 0.5, None, OP.is_gt)
            m2 = sbuf.tile([P, EC, E], BF16, tag="m2")
            nc.vector.tensor_scalar(m2[:], adj_sb[:], 2.0, None, OP.is_equal)
            m3 = sbuf.tile([P, EC, E], BF16, tag="m3")
            nc.vector.tensor_scalar(m3[:], adj_sb[:], 3.0, None, OP.is_equal)

            # ---- Xt via PE transposes (copies on Pool) ----
            Xt_sb = sbuf.tile([P, KC, E], BF16, tag="Xt")
            for kc in range(KC):
                tps = ps_tr.tile([P, E], BF16, tag="tr")
                for ec in range(EC):
                    nc.tensor.transpose(
                        tps[:, ec * P:(ec + 1) * P],
                        X_sb[:, ec, kc * P:(kc + 1) * P], identb[:])
                nc.gpsimd.tensor_copy(Xt_sb[:, kc, :], tps[:])

            # ---- LR rows -> score stacks ----
            pLR = ps_lr.tile([2 * NT, E], F32, tag="lr")
            for kc in range(KC):
                nc.tensor.matmul(pLR[:], U6[:, kc, :, n], Xt_sb[:, kc, :],
                                 start=(kc == 0), stop=(kc == KC - 1))
            sL, sR = stkL[n % 2], stkR[n % 2]
            # l_t -> row 32t+1 of stkL; r_t -> row 32t of stkR
            nc.vector.tensor_copy(sL[1:66:32, :], pLR[0:NT, :])
            nc.vector.tensor_copy(sR[0:65:32, :], pLR[NT:2 * NT, :])

            # ---- per-chunk scores + h ----
            rs = small.tile([P, EC], F32, tag="rs")
            rsr = small.tile([P, EC], F32, tag="rsr")
            E_sb = deep.tile([P, EC, E], BF16, tag="E")
            hs_sb = deep.tile([P, EC, D], BF16, tag="hs")
            for ic in range(EC):
                pv1 = ps_s1.tile([P, E], F32, tag="s1")
                nc.tensor.matmul(
                    pv1[:], sL[0:2, ic * P:(ic + 1) * P].bitcast(F32R),
                    sR[0:2, :].bitcast(F32R), start=True, stop=True)
                pv23 = ps_s23.tile([P, 2, E], F32, tag="s23")
                nc.tensor.matmul(
                    pv23[:, 0, :], sL[32:34, ic * P:(ic + 1) * P].bitcast(F32R),
                    sR[32:34, :].bitcast(F32R), start=True, stop=True)
                nc.tensor.matmul(
                    pv23[:, 1, :], sL[64:66, ic * P:(ic + 1) * P].bitcast(F32R),
                    sR[64:66, :].bitcast(F32R), start=True, stop=True)
                nc.vector.copy_predicated(pv1[:], m2[:, ic, :], pv23[:, 0, :])
                nc.vector.copy_predicated(pv1[:], m3[:, ic, :], pv23[:, 1, :])
                lr_sb = small.tile([P, E], BF16, tag="lrl")
                nc.scalar.activation(lr_sb[:], pv1[:], AF.Prelu, alpha=0.2)
                e1_sb = small.tile([P, E], BF16, tag="e1")
                nc.scalar.activation(e1_sb[:], lr_sb[:], AF.Exp)
                # E' = e1 * (adj>0), rowsum into rs  (DVE 2x)
                nc.vector.tensor_tensor_reduce(
                    E_sb[:, ic, :], e1_sb[:], mz[:, ic, :], 1.0, 0.0,
                    OP.mult, OP.add, accum_out=rs[:, ic:ic + 1])
                nc.vector.reciprocal(rsr[:, ic:ic + 1], rs[:, ic:ic + 1])
                # h chunk; normalization folded into the PSUM->SBUF copy
                pH = ps_big.tile([P, D], F32, tag="big")
                for kc in range(KC):
                    nc.tensor.matmul(pH[:], Xt_sb[:, kc, ic * P:(ic + 1) * P],
                                     Wt2_sb[:, kc, :],
                                     start=(kc == 0), stop=(kc == KC - 1))
                nc.gpsimd.tensor_scalar(hs_sb[:, ic, :], pH[:],
                                        rsr[:, ic:ic + 1], None, OP.mult)

            # ---- out = E'^T @ hs ----
            out_sb = sbuf.tile([P, EC, D], BF16, tag="osb")
            for jc in range(EC):
                pO = ps_big.tile([P, D], F32, tag="big")
                for ic in range(EC):
                    nc.tensor.matmul(pO[:], E_sb[:, ic, jc * P:(jc + 1) * P],
                                     hs_sb[:, ic, :],
                                     start=(ic == 0), stop=(ic == EC - 1))
                nc.scalar.copy(out_sb[:, jc, :], pO[:])
            nc.scalar.dma_start(out[n].rearrange("(c p) d -> p c d", p=P), out_sb[:])

        def body_all(_iv=None):
            run_prep()
            for n in range(NG):
                do_graph(n)

        if reps == 1:
            body_all()
        else:
            with tc.For_i(0, reps, 1) as _iv:
                body_all(_iv)
    return nc


_NC_CACHE = {}
TRACE = False
_LAST = {}


def _get_nc():
    if "nc" not in _NC_CACHE:
        nc = bacc.Bacc("TRN2", target_bir_lowering=False, debug=False)
        build(nc)
        nc.compile()
        _NC_CACHE["nc"] = nc
    return _NC_CACHE["nc"]


def kernel(input_state, adj, entity_mask, query_vec, W_type, a_type,
           qattn_W1, qattn_W2):
    from concourse import bass_utils
    import ml_dtypes
    bf16 = ml_dtypes.bfloat16
    nc = _get_nc()
    x = np.ascontiguousarray(input_state).astype(bf16)
    adjf = np.ascontiguousarray(adj).astype(bf16)
    qv = np.ascontiguousarray(query_vec).astype(bf16)
    Wt = np.ascontiguousarray(W_type).astype(bf16)
    at = np.ascontiguousarray(a_type, dtype=np.float32)
    W1 = np.ascontiguousarray(qattn_W1).astype(bf16)
    W2q = np.ascontiguousarray(qattn_W2).astype(bf16)

    in_maps = []
    for c in range(N_CORES):
        sl = slice(c * NG, (c + 1) * NG)
        in_maps.append({
            "x": x[sl], "adjf": adjf[sl], "qv": qv[sl],
            "Wt": Wt, "at": at, "W1": W1, "W2q": W2q,
        })
    res = bass_utils.run_bass_kernel_spmd(nc, in_maps, core_ids=list(range(N_CORES)),
                                          trace=TRACE, stitch_traces=TRACE)
    _LAST["exec_ns"] = res.exec_time_ns
    _LAST["mean_ns"] = res.mean_exec_time_ns
    _LAST["trace"] = res.instructions_and_trace
    out = np.concatenate([r["out"] for r in res.results], axis=0)
    return out.astype(np.float32)
